# revision 13
# baseline (speedup 1.0000x reference)
"""Trainium2 Bass kernel for nn_GCL (GNN message-passing layer), 8 NeuronCores.

Strategy (edge/graph parallelism, no collectives):
  - Host sorts edges by destination (`row`) and shards them so core k owns all
    edges whose destination node lies in [1250k, 1250(k+1)).  Each core then
    computes the complete aggregate for its node range locally -- the
    all-reduce in the sharding hint is replaced by a partition of the segment
    domain.
  - The first edge-MLP layer is factored through the nodes:
        x @ ew1 = (h @ ew1[:128])[row] + (h @ ew1[128:])[col]
    so each core precomputes P = h@ew1a and Q = h@ew1b + eb1 once (bf16) and
    the per-edge work becomes two 256B gathers + add + silu + one 128x128
    matmul (+ silu) instead of a 256-wide matmul per edge.
  - Gathers use the transposed dma_gather so P[row]/Q[col] arrive
    feature-major ([feat, edges]) -- the layout the second matmul wants as its
    stationary operand -- with zero on-chip transposes in the edge loop.
  - Default precision mode (GCL_SPLIT=1) stores P/Q as bf16 hi+lo pairs and
    computes them from hi/lo-split h and ew1 (3 bf16 matmuls), giving
    ~1e-5 rel-err at 512B gather granularity (the DMA line-rate minimum).
    GCL_SPLIT=0 is a single-bf16 variant (~3e-3 rel-err, ~25% faster).
  - Segment-sum is a one-hot matmul: oh[e, n] = (row_local[e] == n) built on
    DVE via iota + is_equal, then PSUM-accumulated aggT[feat, node] over each
    128-node tile's (padded) edge range.
  - eb2 / nb2 biases (which land on the free dim) are pre-filled into PSUM via
    K=1 matmuls (ones[1,128].T @ bias[1,N]).
  - The node MLP runs inline per 128-node tile in f32.
"""

import numpy as np
import ml_dtypes

BF16 = ml_dtypes.bfloat16

V, E, NF, HID = 10000, 640000, 128, 128
NCORES, VLOC, PT = 8, 1250, 128
NT = (VLOC + PT - 1) // PT          # 10 node tiles per core
VPAD = ((V + PT - 1) // PT) * PT    # 10112 rows of h after padding
NBLK_H = VPAD // PT                 # 79 h blocks
HLOC = NT * PT                      # 1280 local node rows (padded)
NORM = 100.0

_PROG_CACHE = {}


# ----------------------------------------------------------------------------
# host-side preprocessing
# ----------------------------------------------------------------------------

def _preprocess(h, edge_index):
    row = np.asarray(edge_index[0]).astype(np.int64)
    col = np.asarray(edge_index[1]).astype(np.int64)
    order = np.argsort(row, kind="stable")
    row_s, col_s = row[order], col[order]

    ranges = {}
    max_cnt = 0
    for k in range(NCORES):
        for t in range(NT):
            lo = k * VLOC + t * PT
            hi = min(lo + PT, (k + 1) * VLOC)
            s = int(np.searchsorted(row_s, lo))
            e = int(np.searchsorted(row_s, hi))
            ranges[(k, t)] = (s, e, lo)
            max_cnt = max(max_cnt, e - s)
    sb_per_tile = max(1, (max_cnt + 511) // 512)
    eb = sb_per_tile * 512
    ne_pad = NT * eb

    per_core = []
    for k in range(NCORES):
        gp = np.zeros(ne_pad, np.int64)
        gq = np.zeros(ne_pad, np.int64)
        rl = np.full(ne_pad, -1, np.int64)
        orig = np.full(ne_pad, -1, np.int64)
        for t in range(NT):
            s, e, lo = ranges[(k, t)]
            n = e - s
            b0 = t * eb
            gp[b0:b0 + n] = row_s[s:e]
            gq[b0:b0 + n] = col_s[s:e]
            rl[b0:b0 + n] = row_s[s:e] - lo
            orig[b0:b0 + n] = order[s:e]
        # dma_gather index layout: idx i lives at [i % 16, i // 16], then the
        # 16-partition block is replicated 8x across the 128 partitions.
        def wrap16(a):
            w = a.astype(np.int16).reshape(-1, 16).T          # [16, n/16]
            return np.ascontiguousarray(np.tile(w, (8, 1)))   # [128, n/16]
        h_local = np.zeros((HLOC, NF), np.float32)
        h_local[:VLOC] = h[k * VLOC:(k + 1) * VLOC]
        per_core.append(dict(
            gp_idx=wrap16(gp),
            gq_idx=wrap16(gq),
            rl_t=np.ascontiguousarray(rl.reshape(-1, PT).T.astype(np.int32)),
            h_local=h_local,
            orig=orig,
        ))
    return per_core, sb_per_tile


# ----------------------------------------------------------------------------
# device program
# ----------------------------------------------------------------------------

def _build_program(sb_per_tile, split=False, eb2dve=True, ng=None, wbufs=4, mbufs=2, gbufs=2):
    import concourse.bacc as bacc
    import concourse.mybir as mybir
    import concourse.tile as tile
    from concourse.masks import make_identity

    dt = mybir.dt
    AF = mybir.ActivationFunctionType
    ALU = mybir.AluOpType

    EBT = sb_per_tile * 512          # padded edges per node tile
    NE_PAD = NT * EBT
    NCOMP = 2 if split else 1        # bf16 components per value

    nc = bacc.Bacc("TRN2", target_bir_lowering=False, debug=False,
                   enable_asserts=False, num_devices=NCORES)

    # ---- I/O ----
    h_bf = nc.dram_tensor("h_bf", [VPAD, NF], dt.bfloat16, kind="ExternalInput")
    if split:
        h_lo_d = nc.dram_tensor("h_lo", [VPAD, NF], dt.bfloat16, kind="ExternalInput")
        ew1lo_d = nc.dram_tensor("ew1lo", [NF, 2 * HID], dt.bfloat16, kind="ExternalInput")
    h_loc = nc.dram_tensor("h_local", [HLOC, NF], dt.float32, kind="ExternalInput")
    gp_idx = nc.dram_tensor("gp_idx", [128, NE_PAD // 16], dt.int16, kind="ExternalInput")
    gq_idx = nc.dram_tensor("gq_idx", [128, NE_PAD // 16], dt.int16, kind="ExternalInput")
    rl_td = nc.dram_tensor("rl_t", [128, NE_PAD // 128], dt.int32, kind="ExternalInput")
    ew1c_d = nc.dram_tensor("ew1cat", [NF, 2 * HID], dt.bfloat16, kind="ExternalInput")
    eb1c_d = nc.dram_tensor("eb1cat", [1, 2 * HID], dt.float32, kind="ExternalInput")
    ew2_d = nc.dram_tensor("ew2", [HID, HID], dt.float32, kind="ExternalInput")
    eb2r_d = nc.dram_tensor("eb2rep", [1, 512], dt.float32, kind="ExternalInput")
    nw1a_d = nc.dram_tensor("nw1a", [NF, HID], dt.float32, kind="ExternalInput")
    nw1b_d = nc.dram_tensor("nw1b", [HID, HID], dt.float32, kind="ExternalInput")
    nb1_d = nc.dram_tensor("nb1c", [HID, 1], dt.float32, kind="ExternalInput")
    nw2_d = nc.dram_tensor("nw2", [HID, NF], dt.float32, kind="ExternalInput")
    nb2_d = nc.dram_tensor("nb2r", [1, NF], dt.float32, kind="ExternalInput")

    mij_out = nc.dram_tensor("mij_out", [NE_PAD, HID], dt.float32, kind="ExternalOutput")
    out_loc = nc.dram_tensor("out_loc", [HLOC, NF], dt.float32, kind="ExternalOutput")

    with tile.TileContext(nc) as tc:
        import contextlib
        with contextlib.ExitStack() as ctx:
            cpool = ctx.enter_context(tc.tile_pool(name="const", bufs=1))
            dpool = ctx.enter_context(tc.tile_pool(name="dram", bufs=1, space="DRAM"))

            # ---- constants / weights in SBUF ----
            ones1 = cpool.tile([1, 128], dt.float32)
            nc.gpsimd.memset(ones1[:], 1.0)
            ident = cpool.tile([128, 128], dt.float32)
            make_identity(nc, ident[:])
            iota4 = cpool.tile([128, 4, 128], dt.int32)
            nc.gpsimd.iota(iota4[:], pattern=[[0, 4], [1, 128]], base=0,
                           channel_multiplier=0)

            ew1cat = cpool.tile([NF, 2 * HID], dt.bfloat16)
            if split:
                ew1lo = cpool.tile([NF, 2 * HID], dt.bfloat16)
                nc.sync.dma_start(out=ew1lo[:], in_=ew1lo_d.ap())
            eb1cat = cpool.tile([1, 2 * HID], dt.float32)
            ew2 = cpool.tile([HID, HID], dt.float32)
            eb2rep = cpool.tile([1, 512], dt.float32)
            nw1a = cpool.tile([NF, HID], dt.float32)
            nw1b = cpool.tile([HID, HID], dt.float32)
            nb1 = cpool.tile([HID, 1], dt.float32)
            nw2 = cpool.tile([HID, NF], dt.float32)
            nb2 = cpool.tile([1, NF], dt.float32)
            for sb, dr in [(ew1cat, ew1c_d), (eb1cat, eb1c_d),
                           (ew2, ew2_d), (eb2rep, eb2r_d), (nw1a, nw1a_d),
                           (nw1b, nw1b_d), (nb1, nb1_d), (nw2, nw2_d),
                           (nb2, nb2_d)]:
                nc.sync.dma_start(out=sb[:], in_=dr.ap())

            gp_sb = cpool.tile([128, NE_PAD // 16], dt.int16)
            gq_sb = cpool.tile([128, NE_PAD // 16], dt.int16)
            rl_sb = cpool.tile([128, NE_PAD // 128], dt.int32)
            nc.sync.dma_start(out=gp_sb[:], in_=gp_idx.ap())
            nc.sync.dma_start(out=gq_sb[:], in_=gq_idx.ap())
            nc.sync.dma_start(out=rl_sb[:], in_=rl_td.ap())

            if eb2dve:
                # one-time eb2 broadcast tile [128, 512] via K=1 matmul
                eb2bc = cpool.tile([128, 4, 128], dt.float32)
                with tc.tile_pool(name="ebps", bufs=1, space="PSUM") as ebps:
                    ps0 = ebps.tile([128, 512], dt.float32, space="PSUM")
                    nc.tensor.matmul(ps0[:], lhsT=ones1[:], rhs=eb2rep[:],
                                     start=True, stop=True)
                    nc.vector.tensor_copy(eb2bc[:].rearrange("p b f -> p (b f)"), ps0[:])

            # ---- phase A: P/Q precompute -> pq_dram ----
            # layout per node row: [P (NCOMP*128) | Q (NCOMP*128)] bf16
            pq_dram = dpool.tile([VPAD, NCOMP * 2 * HID], dt.bfloat16)

            with tc.tile_pool(name="phA", bufs=3) as pa, \
                 tc.tile_pool(name="phA_ht", bufs=1) as pht, \
                 tc.tile_pool(name="phA_ps", bufs=2, space="PSUM") as pps:
                hT = pht.tile([128, VPAD], dt.bfloat16)
                nc.sync.dma_start_transpose(hT[:], h_bf.ap())
                if split:
                    hTlo = pht.tile([128, VPAD], dt.bfloat16, tag="hTlo")
                    nc.sync.dma_start_transpose(hTlo[:], h_lo_d.ap())
                for b in range(NBLK_H):
                    ps = pps.tile([128, 2, 128], dt.float32, space="PSUM", tag="pq")
                    flat = ps[:].rearrange("p b f -> p (b f)")
                    nc.tensor.matmul(flat, lhsT=ones1[:], rhs=eb1cat[:],
                                     start=True, stop=False, skip_group_check=True)
                    hTb = hT[:, b * 128:(b + 1) * 128]
                    if not split:
                        nc.tensor.matmul(flat, lhsT=hTb, rhs=ew1cat[:],
                                         start=False, stop=True, skip_group_check=True)
                    else:
                        nc.tensor.matmul(flat, lhsT=hTb, rhs=ew1cat[:],
                                         start=False, stop=False, skip_group_check=True)
                        nc.tensor.matmul(flat, lhsT=hTb, rhs=ew1lo[:],
                                         start=False, stop=False, skip_group_check=True)
                        nc.tensor.matmul(flat, lhsT=hTlo[:, b * 128:(b + 1) * 128],
                                         rhs=ew1cat[:],
                                         start=False, stop=True, skip_group_check=True)
                    if not split:
                        pq = pa.tile([128, 2, 128], dt.bfloat16, tag="pq_sb")
                        nc.vector.tensor_copy(pq[:], ps[:])
                    else:
                        # hi/lo split: [P_hi P_lo Q_hi Q_lo]
                        pq = pa.tile([128, 4, 128], dt.bfloat16, tag="pq_sb")
                        hi = pa.tile([128, 2, 128], dt.bfloat16, tag="hi_sb")
                        nc.vector.tensor_copy(hi[:], ps[:])
                        nc.vector.tensor_copy(pq[:, 0, :], hi[:, 0, :])
                        nc.vector.tensor_copy(pq[:, 2, :], hi[:, 1, :])
                        lo = pa.tile([128, 2, 128], dt.float32, tag="lo_sb")
                        nc.vector.tensor_tensor(out=lo[:], in0=ps[:], in1=hi[:],
                                                op=ALU.subtract)
                        nc.vector.tensor_copy(pq[:, 1, :], lo[:, 0, :])
                        nc.vector.tensor_copy(pq[:, 3, :], lo[:, 1, :])
                    nc.sync.dma_start(out=pq_dram[b * 128:(b + 1) * 128, :],
                                      in_=pq[:].rearrange("p b f -> p (b f)"))

            # ---- phase B/C: edges + node MLP ----
            with tc.tile_pool(name="gat", bufs=gbufs) as gpool, \
                 tc.tile_pool(name="work", bufs=wbufs) as wpool, \
                 tc.tile_pool(name="ntile", bufs=2) as npool, \
                 tc.tile_pool(name="ps_m", bufs=mbufs, space="PSUM") as psm, \
                 tc.tile_pool(name="ps_agg", bufs=2, space="PSUM") as psagg, \
                 tc.tile_pool(name="ps_n", bufs=1, space="PSUM") as psn:
                NG = ng if ng is not None else (2 if split else 1)  # gather chunks per tile
                sbc = (sb_per_tile + NG - 1) // NG   # superblocks per chunk
                for t in range(NT):
                    # gathers for this node tile (feature-major, bf16)
                    PW = NCOMP * HID
                    pgs, qgs = [], []
                    for g in range(NG):
                        nsb = min(sbc, sb_per_tile - g * sbc)
                        nidx = nsb * 512
                        i0 = t * (EBT // 16) + g * (sbc * 512 // 16)
                        pg = gpool.tile([128, NCOMP, nidx], dt.bfloat16, tag="pg")
                        qg = gpool.tile([128, NCOMP, nidx], dt.bfloat16, tag="qg")
                        nc.gpsimd.dma_gather(
                            out_ap=pg[:], in_ap=pq_dram[:, 0:PW],
                            idxs_ap=gp_sb[:, i0:i0 + nidx // 16],
                            num_idxs=nidx, num_idxs_reg=nidx, elem_size=PW,
                            elem_step=2 * PW, transpose=True, single_packet=False)
                        nc.gpsimd.dma_gather(
                            out_ap=qg[:], in_ap=pq_dram[:, PW:2 * PW],
                            idxs_ap=gq_sb[:, i0:i0 + nidx // 16],
                            num_idxs=nidx, num_idxs_reg=nidx, elem_size=PW,
                            elem_step=2 * PW, transpose=True, single_packet=False)
                        pgs.append(pg); qgs.append(qg)

                    agg = psagg.tile([128, 128], dt.float32, space="PSUM", tag="agg")
                    for s in range(sb_per_tile):
                        g = s // sbc
                        pg, qg = pgs[g], qgs[g]
                        e0 = (s - g * sbc) * 512
                        sbg = t * sb_per_tile + s      # global superblock idx
                        z = wpool.tile([128, 512], dt.float32, tag="z")
                        if not split:
                            nc.vector.tensor_tensor(out=z[:], in0=pg[:, 0, e0:e0 + 512],
                                                    in1=qg[:, 0, e0:e0 + 512], op=ALU.add)
                        else:
                            zh = wpool.tile([128, 512], dt.float32, tag="zh")
                            nc.vector.tensor_tensor(out=zh[:], in0=pg[:, 0, e0:e0 + 512],
                                                    in1=qg[:, 0, e0:e0 + 512], op=ALU.add)
                            zl = wpool.tile([128, 512], dt.float32, tag="zl")
                            nc.vector.tensor_tensor(out=zl[:], in0=pg[:, 1, e0:e0 + 512],
                                                    in1=qg[:, 1, e0:e0 + 512], op=ALU.add)
                            nc.vector.tensor_tensor(out=z[:], in0=zh[:], in1=zl[:],
                                                    op=ALU.add)
                        sfm = wpool.tile([128, 512], dt.float32, tag="sfm")
                        nc.scalar.activation(sfm[:], z[:], AF.Silu)
                        oh = wpool.tile([128, 4, 128], dt.float32, tag="oh")
                        nc.vector.tensor_tensor(
                            out=oh[:], in0=iota4[:],
                            in1=rl_sb[:, sbg * 4:sbg * 4 + 4].to_broadcast([128, 4, 128]),
                            op=ALU.is_equal)
                        psb = psm.tile([128, 4, 128], dt.float32, space="PSUM", tag="m")
                        if eb2dve:
                            for b in range(4):
                                nc.tensor.matmul(psb[:, b, :],
                                                 lhsT=sfm[:, b * 128:(b + 1) * 128],
                                                 rhs=ew2[:], start=True, stop=True,
                                                 skip_group_check=True)
                            nc.vector.tensor_tensor(
                                out=psb[:].rearrange("p b f -> p (b f)"),
                                in0=psb[:].rearrange("p b f -> p (b f)"),
                                in1=eb2bc[:].rearrange("p b f -> p (b f)"), op=ALU.add)
                        else:
                            nc.tensor.matmul(psb[:].rearrange("p b f -> p (b f)"),
                                             lhsT=ones1[:], rhs=eb2rep[:],
                                             start=True, stop=False, skip_group_check=True)
                            for b in range(4):
                                nc.tensor.matmul(psb[:, b, :],
                                                 lhsT=sfm[:, b * 128:(b + 1) * 128],
                                                 rhs=ew2[:], start=False, stop=True,
                                                 skip_group_check=True)
                        mij = wpool.tile([128, 4, 128], dt.float32, tag="mij")
                        nc.scalar.activation(mij[:], psb[:], AF.Silu)
                        dst = mij_out.ap()[sbg * 512:(sbg + 1) * 512, :].rearrange(
                            "(b p) f -> p b f", p=128)
                        nc.sync.dma_start(out=dst, in_=mij[:])
                        for b in range(4):
                            nc.tensor.matmul(
                                agg[:], lhsT=mij[:, b, :], rhs=oh[:, b, :],
                                start=(s == 0 and b == 0),
                                stop=(s == sb_per_tile - 1 and b == 3),
                                skip_group_check=True)

                    # aggT -> sbuf (f32); /NORM is folded into nw1b on host
                    aggs = npool.tile([128, 128], dt.float32, tag="aggs")
                    nc.vector.tensor_copy(aggs[:], agg[:])

                    # ---- node MLP for tile t ----
                    htile = npool.tile([128, 128], dt.float32, tag="htile")
                    nc.sync.dma_start(out=htile[:],
                                      in_=h_loc.ap()[t * 128:(t + 1) * 128, :])
                    ps_ht = psn.tile([128, 128], dt.float32, space="PSUM", tag="ht")
                    nc.tensor.transpose(ps_ht[:], htile[:], ident[:])
                    hTs = npool.tile([128, 128], dt.float32, tag="hTs")
                    nc.vector.tensor_copy(hTs[:], ps_ht[:])
                    ps_n1 = psn.tile([128, 128], dt.float32, space="PSUM", tag="n1")
                    nc.tensor.matmul(ps_n1[:], lhsT=nw1a[:], rhs=hTs[:],
                                     start=True, stop=False)
                    nc.tensor.matmul(ps_n1[:], lhsT=nw1b[:], rhs=aggs[:],
                                     start=False, stop=True)
                    s1t = npool.tile([128, 128], dt.float32, tag="s1t")
                    nc.scalar.activation(s1t[:], ps_n1[:], AF.Silu, bias=nb1[:])
                    ps_n2 = psn.tile([128, 128], dt.float32, space="PSUM", tag="n2")
                    nc.tensor.matmul(ps_n2[:], lhsT=ones1[:], rhs=nb2[:],
                                     start=True, stop=False, skip_group_check=True)
                    nc.tensor.matmul(ps_n2[:], lhsT=s1t[:], rhs=nw2[:],
                                     start=False, stop=True, skip_group_check=True)
                    ot = npool.tile([128, 128], dt.float32, tag="ot")
                    nc.vector.tensor_tensor(out=ot[:], in0=ps_n2[:], in1=htile[:],
                                            op=ALU.add)
                    nc.sync.dma_start(out=out_loc.ap()[t * 128:(t + 1) * 128, :],
                                      in_=ot[:])

    nc.compile()
    return nc


# ----------------------------------------------------------------------------
# entry point
# ----------------------------------------------------------------------------

def kernel(h, edge_index, ew1, eb1, ew2, eb2, nw1, nb1, nw2, nb2):
    from concourse.bass_utils import run_bass_kernel_spmd

    h = np.asarray(h, np.float32)
    ew1 = np.asarray(ew1, np.float32)
    eb1 = np.asarray(eb1, np.float32)
    ew2 = np.asarray(ew2, np.float32)
    eb2 = np.asarray(eb2, np.float32)
    nw1 = np.asarray(nw1, np.float32)
    nb1 = np.asarray(nb1, np.float32)
    nw2 = np.asarray(nw2, np.float32)
    nb2 = np.asarray(nb2, np.float32)

    per_core, sb_per_tile = _preprocess(h, edge_index)

    import os
    split = bool(int(os.environ.get("GCL_SPLIT", "1")))
    ng = int(os.environ.get("GCL_NG", "6"))
    key = (sb_per_tile, split, ng)
    if key not in _PROG_CACHE:
        _PROG_CACHE[key] = _build_program(sb_per_tile, split=split,
                                          eb2dve=not split, ng=ng, gbufs=3)
    nc = _PROG_CACHE[key]

    h_pad = np.zeros((VPAD, NF), np.float32)
    h_pad[:V] = h
    eb1cat = np.zeros((1, 2 * HID), np.float32)
    eb1cat[0, HID:] = eb1
    ew1c = np.ascontiguousarray(np.concatenate([ew1[:NF], ew1[NF:]], axis=1))
    shared = dict(
        h_bf=h_pad.astype(BF16),
        ew1cat=ew1c.astype(BF16),
        eb1cat=eb1cat,
        ew2=np.ascontiguousarray(ew2),
        eb2rep=np.ascontiguousarray(np.tile(eb2.reshape(1, HID), (1, 4))),
        nw1a=np.ascontiguousarray(nw1[:NF]),
        nw1b=np.ascontiguousarray(nw1[NF:] / NORM),
        nb1c=np.ascontiguousarray(nb1.reshape(HID, 1)),
        nw2=np.ascontiguousarray(nw2),
        nb2r=np.ascontiguousarray(nb2.reshape(1, NF)),
    )
    if split:
        shared["h_lo"] = (h_pad - shared["h_bf"].astype(np.float32)).astype(BF16)
        shared["ew1lo"] = (ew1c - shared["ew1cat"].astype(np.float32)).astype(BF16)
    in_maps = []
    for k in range(NCORES):
        m = dict(shared)
        m["h_local"] = per_core[k]["h_local"]
        m["gp_idx"] = per_core[k]["gp_idx"]
        m["gq_idx"] = per_core[k]["gq_idx"]
        m["rl_t"] = per_core[k]["rl_t"]
        in_maps.append(m)

    trace = bool(int(os.environ.get("GCL_TRACE", "0")))
    res = run_bass_kernel_spmd(nc, in_maps, core_ids=list(range(NCORES)),
                               trace=trace)
    global LAST_RESULTS
    LAST_RESULTS = res

    out = np.empty((V, NF), np.float32)
    mij = np.empty((E, HID), np.float32)
    for k in range(NCORES):
        r = res.results[k]
        out[k * VLOC:(k + 1) * VLOC] = r["out_loc"][:VLOC]
        orig = per_core[k]["orig"]
        valid = orig >= 0
        mij[orig[valid]] = r["mij_out"][valid]
    return out, mij


LAST_RESULTS = None
